# revision 41
# baseline (speedup 1.0000x reference)
"""Multi-head attention (B=4, S=2048, D=1024, H=16) on 8 trn2 NeuronCores.

Sharding: core c = (batch b, head-group g) with b in 0..3, g in 0..1.
Each core computes 8 heads of one batch; the two cores of a batch produce
partial output projections that the host sums.

Layouts (feature dim on SBUF partitions, no on-device transposes):
  Q^T/K^T [d, s], V [s, d], scores^T [k, q], o^T [d, q], y^T [out, q].

Schedule (v2): pair-major iteration (pair, qc) with chase-mode PV (PV for
kt trails the exp by one k-tile inside the same iteration).  The softmax
exp on the Scalar engine is the pacing resource, so the prologue is minimal
(K for pair 0 + the first Q chunk) and all remaining QKV projection groups
drain into the main loop against emission deadlines.  PV runs as four
concurrent 32-column col-tiled matmuls (one PSUM bank); softmax denominators
come from ones-vector matmuls batched two k-tiles per 4-slot col-tiled pass.
"""
import math

import numpy as np
import ml_dtypes

import concourse.bass as bass
import concourse.mybir as mybir
import concourse.tile as tile
from concourse import bacc
from concourse.bass_utils import run_bass_kernel_spmd

B, S, D, H = 4, 2048, 1024, 16
DK = D // H              # 64
NCORES = 8
HG = 2                   # head groups (tensor-parallel axis)
HPG = H // HG            # 8 heads per core
HD = HPG * DK            # 512 head-dim features per core
PAIRS = HPG // 2         # 4 head pairs (2 heads row-packed per PE pass)
P = 128
QC = 512                 # q-chunk (matmul moving free dim)
NQC = S // QC            # 4
NKT = S // P             # 16 k-tiles
FK = D // P              # 8 feature c-tiles for projections
TC = 512                 # token chunk for QKV phase
NTC = S // TC            # 4

F32 = mybir.dt.float32
BF16 = mybir.dt.bfloat16
I16 = mybir.dt.int16

# Schraudolph bf16 exp constants (DVE offload): bitcast(int16(A*x + B)) ~ e^x
SCH_A = 128.0 / math.log(2.0)
SCH_B = 127.0 * 128.0 - 5.7666

LAST_EXEC_NS = None


def _build(apply_mask: bool, qkv_bias: bool, n_dve_exp: int = 0):
    nc = bacc.Bacc("TRN2", debug=False, num_devices=NCORES)
    xT = nc.declare_dram_parameter("xT", [D, S], BF16, isOutput=False)
    wqkv = nc.declare_dram_parameter("wqkv", [D, 3 * HD], BF16, isOutput=False)
    wo = nc.declare_dram_parameter("wo", [HD, D], BF16, isOutput=False)
    yT = nc.declare_dram_parameter("yT", [D, S], F32, isOutput=True)
    if apply_mask:
        maskT = nc.declare_dram_parameter("maskT", [S, S], F32, isOutput=False)
    if qkv_bias:
        qkb = nc.declare_dram_parameter("qkb", [2, HD], F32, isOutput=False)
        vb = nc.declare_dram_parameter("vb", [HD], F32, isOutput=False)

    xT_r = xT.rearrange("(fo p) s -> p fo s", p=P)       # [128, 8, 2048]
    wqkv_r = wqkv.rearrange("(fo p) n -> p fo n", p=P)   # [128, 8, 1536]
    wo_r = wo.rearrange("(co p) n -> p co n", p=P)       # [128, 4, 1024]
    yT_r = yT.rearrange("(oo p) s -> p oo s", p=P)       # [128, 8, 2048]

    # which k-tiles of each iteration run their exp on the vector engine
    dve_kts = set()
    if n_dve_exp > 0:
        step = NKT / n_dve_exp
        dve_kts = {min(NKT - 1, int((i + 0.5) * step)) for i in range(n_dve_exp)}

    with tile.TileContext(nc) as tc:
        with tc.tile_pool(name="persist", bufs=1) as persist, \
             tc.tile_pool(name="work", bufs=2) as work, \
             tc.tile_pool(name="small", bufs=1) as small, \
             tc.tile_pool(name="phat", bufs=12) as phatp, \
             tc.tile_pool(name="ps_sc", bufs=2, space="PSUM") as ps_sc, \
             tc.tile_pool(name="ps_pv", bufs=1, space="PSUM") as ps_pv, \
             tc.tile_pool(name="ps_qkv", bufs=2, space="PSUM") as ps_qkv:

            QT = persist.tile([P, PAIRS, S], BF16)        # 16KB/part
            KTt = persist.tile([P, PAIRS, S], BF16)       # 16KB/part
            V = persist.tile([P, NKT, HPG * (DK + 1)], BF16)  # 16.25KB/part
            wo_t = persist.tile([P, HD // P, D], BF16)    # 8KB/part
            o_tiles = [persist.tile([P, HD // P, QC], BF16, name=f"o_sb{qc}")
                       for qc in range(NQC)]              # 4x4KB/part

            # --- t=0 helpers: exp-table preload + PE HAM warm-up ----------
            dummy_w = persist.tile([P, 16], BF16)
            nc.vector.memset(dummy_w, 0.0)
            # ones columns of V (softmax-denominator trick)
            for h in range(HPG):
                nc.vector.memset(V[:, :, h * (DK + 1) + DK], 1.0)
            dact_in = small.tile([1, 2], F32, tag="dact_i")
            dact_out = small.tile([1, 2], F32, tag="dact_o")
            nc.vector.memset(dact_in, 0.0)
            nc.scalar.activation(dact_out, dact_in,
                                 mybir.ActivationFunctionType.Exp)
            # enough dummy matmuls to keep the PE HAM busy across the input
            # DMA wait, so the first real projections run at full clock
            ps_warm = ps_qkv.tile([P, QC], F32, tag="qkv", name="ps_warm")
            for i in range(256):
                nc.tensor.matmul(ps_warm[0:16, 0:16], dummy_w, dummy_w,
                                 start=True, stop=True)

            if qkv_bias:
                qkb_t = persist.tile([P, 2, PAIRS], F32)
                nc.sync.dma_start(
                    qkb_t, qkb.rearrange("t (pr p) -> p t pr", p=P))
                vb_bc = persist.tile([P, HD], F32)
                nc.sync.dma_start(vb_bc, vb[None, :].partition_broadcast(P))

            # --- input DMA, ordered so the prologue's needs land first ----
            x_ko, w_ko = [], []
            for ko in range(FK):
                xk = persist.tile([P, S], BF16, name=f"x_ko{ko}")   # 4KB each
                x_ko.append(xk)
                wk = persist.tile([P, 3 * HD], BF16, name=f"w_ko{ko}")  # 3KB
                w_ko.append(wk)
            # DMA priority order: the prologue needs pair-0's K/Q weight
            # columns and the first two token chunks of x before anything
            # else; V-weights next (V groups drain from iteration 0).
            for ko in range(FK):
                nc.sync.dma_start(w_ko[ko][:, HD:HD + P],
                                  wqkv_r[:, ko, HD:HD + P])
            for ko in range(FK):
                nc.sync.dma_start(x_ko[ko][:, 0:TC], xT_r[:, ko, 0:TC])
            for ko in range(FK):
                nc.sync.dma_start(w_ko[ko][:, 0:P], wqkv_r[:, ko, 0:P])
            for ko in range(FK):
                nc.sync.dma_start(w_ko[ko][:, 2 * HD:3 * HD],
                                  wqkv_r[:, ko, 2 * HD:3 * HD])
            for ko in range(FK):
                nc.sync.dma_start(x_ko[ko][:, TC:2 * TC],
                                  xT_r[:, ko, TC:2 * TC])
            for ko in range(FK):
                nc.sync.dma_start(w_ko[ko][:, HD + P:2 * HD],
                                  wqkv_r[:, ko, HD + P:2 * HD])
            for ko in range(FK):
                nc.sync.dma_start(w_ko[ko][:, P:HD], wqkv_r[:, ko, P:HD])
            for tcx in range(2, NTC):
                tsl = slice(tcx * TC, (tcx + 1) * TC)
                for ko in range(FK):
                    nc.sync.dma_start(x_ko[ko][:, tsl], xT_r[:, ko, tsl])
            nc.sync.dma_start(wo_t, wo_r)

            # --- emit helpers --------------------------------------------
            qk_seq = [0]
            in_main = [False]

            def emit_qk_group(which, pair, tcix, part=None, hold={}):
                # which: 0=Q, 1=K.  Writes QT/KTt[:, pair, token-chunk].
                # part=0/1 emits only half the accumulation chain (finer
                # interleaving for main-loop drains); part=None does both.
                tsl = slice(tcix * TC, (tcix + 1) * TC)
                base = 0 if which == 0 else HD
                if part in (0, None):
                    if not in_main[0] and qk_seq[0] % 2 == 0:
                        psqk = ps_sc.tile([P, 2 * QC], F32, tag="scores",
                                          name="psqk_s")[:, :TC]
                    else:
                        psqk = ps_qkv.tile([P, QC], F32, tag="qkv",
                                           name="psqk")
                    qk_seq[0] += 1
                    hold[(which, pair, tcix)] = psqk
                else:
                    psqk = hold.pop((which, pair, tcix))
                msl = slice(base + pair * P, base + (pair + 1) * P)
                kos = (range(FK) if part is None else
                       range(0, FK // 2) if part == 0 else range(FK // 2, FK))
                for ko in kos:
                    nc.tensor.matmul(
                        psqk, w_ko[ko][:, msl], x_ko[ko][:, tsl],
                        start=(ko == 0), stop=(ko == FK - 1))
                if part == 0:
                    return
                dst = (QT if which == 0 else KTt)[:, pair, tsl]
                if qkv_bias:
                    nc.vector.tensor_scalar_add(
                        dst, psqk, qkb_t[:, which, pair, None])
                else:
                    nc.vector.tensor_copy(dst, psqk)

            def emit_v_group(kt):
                psv = ps_qkv.tile([P, QC], F32, tag="qkv", name="psv")
                for ko in range(FK):
                    nc.tensor.matmul(
                        psv, x_ko[ko][:, kt * P:(kt + 1) * P],
                        w_ko[ko][:, 2 * HD:3 * HD],
                        start=(ko == 0), stop=(ko == FK - 1))
                vdst = V[:, kt, :].rearrange(
                    "p (h w) -> p h w", h=HPG)[:, :, :DK]
                vsrc = psv.rearrange("p (h w) -> p h w", h=HPG)
                if qkv_bias:
                    nc.vector.tensor_add(
                        vdst, vsrc, vb_bc.rearrange("p (h w) -> p h w", h=HPG))
                else:
                    nc.vector.tensor_copy(vdst, vsrc)

            def emit_proj_group(qc, oc, epilogue=False):
                qsl = slice(qc * QC, (qc + 1) * QC)
                if epilogue:
                    psy = ps_sc.tile([P, 2 * QC], F32, tag="scores",
                                     name="psy_s")[:, :QC]
                else:
                    psy = ps_qkv.tile([P, QC], F32, tag="qkv", name="psy")
                for c in range(HD // P):
                    nc.tensor.matmul(
                        psy, wo_t[:, c, oc * P:(oc + 1) * P],
                        o_tiles[qc][:, c, :],
                        start=(c == 0), stop=(c == HD // P - 1))
                yst = work.tile([P, QC], F32, tag="y")
                nc.vector.tensor_copy(yst, psy)
                nc.sync.dma_start(yT_r[:, oc, qsl], yst)

            def emit_pv(pair, phs, kt, pso):
                for half in range(2):
                    hh = 2 * pair + half
                    vcol = slice(hh * (DK + 1), (hh + 1) * (DK + 1))
                    nc.tensor.matmul(
                        pso[0:DK + 1, half * QC:(half + 1) * QC],
                        V[:, kt, vcol],
                        phs[kt][:, half * QC:(half + 1) * QC],
                        start=(kt == 0), stop=(kt == NKT - 1))

            def emit_tail(pso, qc, pair):
                # copy accumulator out of PSUM quickly (pso bufs=1 — next
                # iteration's PV is gated on these reads), then normalize
                # off-critical-path from the SBUF copy.  Full-partition
                # copies: rows 65..127 of pso are never written and unused.
                o_hat = small.tile([DK, 2, QC], F32, tag="o_hat")
                l0 = small.tile([1, 2, QC], F32, tag="l0")
                for half in range(2):
                    hsl = slice(half * QC, (half + 1) * QC)
                    nc.vector.tensor_copy(l0[:, half, :],
                                          pso[DK:DK + 1, hsl])
                    nc.vector.tensor_copy(o_hat[:, half, :],
                                          pso[0:DK, hsl])
                for half in range(2):
                    r_sb = small.tile([1, QC], F32, tag=f"r{half}")
                    nc.vector.reciprocal_approx_fast(r_sb, l0[:, half, :])
                    r_bc = small.tile([DK, QC], F32, tag=f"rbc{half}")
                    nc.gpsimd.partition_broadcast(r_bc, r_sb)
                    nc.vector.tensor_mul(
                        o_tiles[qc][half * DK:(half + 1) * DK, pair, :],
                        o_hat[:, half, :], r_bc)

            # --- iteration order: staircase over (pair, qc) ---------------
            # spreads both the K-projection deadlines (pair axis) and the
            # output-projection work (qc axis) across the whole run.
            order = []
            for d in range(PAIRS + NQC - 1):
                for pr in range(PAIRS):
                    if 0 <= d - pr < NQC:
                        order.append((pr, d - pr))
            pos = {pq: i for i, pq in enumerate(order)}
            first_pos = {pr: min(pos[(pr, q)] for q in range(NQC))
                         for pr in range(PAIRS)}
            last_qc_pos = {q: max(pos[(pr, q)] for pr in range(PAIRS))
                           for q in range(NQC)}

            # --- prologue: K(pair0, chunks 0-1) + Q(pair0, chunk0) --------
            emit_qk_group(1, 0, 0)
            emit_qk_group(1, 0, 1)
            emit_qk_group(0, 0, 0)
            for kt in range(0, 6):
                emit_v_group(kt)

            # --- pending QKV/proj work with emission deadlines ------------
            # deadline = global substep (iter*NKT + kt) BEFORE which the
            # group must be emitted (with a few substeps of completion
            # margin).
            pending = []
            for kt in range(6, NKT):
                pending.append((max(0, kt - 2),
                                (lambda kt=kt: emit_v_group(kt))))
            for pr in range(PAIRS):
                for tcx in range(NTC):
                    if pr == 0 and tcx < 2:
                        continue
                    pending.append(
                        (max(0, first_pos[pr] * NKT + tcx * 4 - 4),
                         (lambda p=pr, t=tcx: emit_qk_group(1, p, t))))
                for tcx in range(NTC):
                    if pr == 0 and tcx == 0:
                        continue
                    pending.append(
                        (max(0, pos[(pr, tcx)] * NKT - 4),
                         (lambda p=pr, t=tcx: emit_qk_group(0, p, t))))
            pending.sort(key=lambda it: it[0])

            HORIZON = 8

            def drain(s, opportunistic):
                n = 0
                while pending and pending[0][0] <= s:
                    pending.pop(0)[1]()
                    n += 1
                while (opportunistic and n < 2 and pending
                       and pending[0][0] <= s + HORIZON):
                    pending.pop(0)[1]()
                    n += 1

            # --- main loop: staircase order, software-pipelined PV --------
            # scores for kt and kt+1 are emitted back-to-back so each
            # half's LDWEIGHTS hides under the other half's matmul.  PV
            # matmuls trail through a FIFO with ~4-6 k-tiles of lag and umm
            # flow across iteration boundaries, so every block leads with
            # the scores feeding the exp stream.
            in_main[0] = True
            from collections import deque
            pv_fifo = deque()
            pv_depth = [0]

            def pump_pv(limit=2, min_depth=4):
                n = 0
                while pv_fifo and n < limit and pv_depth[0] > min_depth:
                    kind, fn = pv_fifo.popleft()
                    fn()
                    if kind == "pv":
                        pv_depth[0] -= 1
                        n += 1

            for it, (pair, qc) in enumerate(order):
                qsl = slice(qc * QC, (qc + 1) * QC)
                if apply_mask:
                    mt = work.tile([P, NKT, QC], F32, tag="mask")
                    nc.sync.dma_start(
                        mt, maskT.rearrange(
                            "(ko p) q -> p ko q", p=P)[:, :, qsl])
                psoh = {}

                def get_pso(h=psoh):
                    if "t" not in h:
                        h["t"] = ps_pv.tile([P, 2 * QC], F32, tag="pv",
                                            name="pso")
                    return h["t"]

                phs = {}

                def emit_scores_exp(kt):
                    ksl = slice(kt * P, (kt + 1) * P)
                    pss = ps_sc.tile([P, 2 * QC], F32, tag="scores",
                                     name="pss")
                    nc.tensor.matmul(
                        pss[:, 0:QC], KTt[0:DK, pair, ksl],
                        QT[0:DK, pair, qsl], start=True, stop=True)
                    nc.tensor.matmul(
                        pss[:, QC:2 * QC], KTt[DK:P, pair, ksl],
                        QT[DK:P, pair, qsl], start=True, stop=True)
                    if apply_mask:
                        nc.vector.tensor_add(
                            pss[:, 0:QC], pss[:, 0:QC], mt[:, kt])
                        nc.vector.tensor_add(
                            pss[:, QC:2 * QC], pss[:, QC:2 * QC], mt[:, kt])
                    return pss

                for kt2 in range(0, NKT, 2):
                    drain(it * NKT + kt2 + 1, opportunistic=True)
                    pss0 = emit_scores_exp(kt2)
                    pss1 = emit_scores_exp(kt2 + 1)
                    for kt, pss in ((kt2, pss0), (kt2 + 1, pss1)):
                        ph = phatp.tile([P, 2 * QC], BF16, tag="ph",
                                        name="ph")
                        phs[kt] = ph
                        if kt in dve_kts:
                            nc.vector.tensor_scalar(
                                ph.bitcast(I16), pss, SCH_A, SCH_B,
                                mybir.AluOpType.mult, mybir.AluOpType.add)
                        else:
                            nc.scalar.activation(
                                ph, pss, mybir.ActivationFunctionType.Exp)
                    for kt in (kt2, kt2 + 1):
                        pv_fifo.append(
                            ("pv", (lambda pr=pair, ph=phs, k=kt, gp=get_pso:
                                    emit_pv(pr, ph, k, gp()))))
                        pv_depth[0] += 1
                    pump_pv()
                def tail_and_proj(gp=get_pso, q=qc, pr=pair):
                    emit_tail(gp(), q, pr)
                    if pr == PAIRS - 1:
                        pending.extend(
                            ((last_qc_pos[q] + 1) * NKT + 6 + 2 * oc,
                             (lambda q=q, oc=oc, ep=(q == NQC - 1):
                              emit_proj_group(q, oc, epilogue=ep)))
                            for oc in range(D // P))
                        pending.sort(key=lambda x: x[0])

                pv_fifo.append(("tail", tail_and_proj))

            while pv_fifo:
                pv_fifo.popleft()[1]()
            while pending:
                pending.pop(0)[1]()

    nc.finalize()
    return nc


# --------------------------------------------------------------------------
# NTFF profiling shim (only used when kernel(..., _trace=True); provides
# antenv.axon_hooks so run_bass_kernel_spmd can capture profiles under axon).
def _install_ntff_shim():
    import contextlib, ctypes, sys, types
    try:
        import antenv.axon_hooks  # noqa: F401
        return
    except ImportError:
        pass
    so = "/opt/axon/libaxon_pjrt.so"
    try:
        lib = ctypes.CDLL(so)
    except OSError:
        return
    if not hasattr(lib, "axon_start_nrt_profile"):
        return
    lib.axon_start_nrt_profile.argtypes = [
        ctypes.POINTER(ctypes.c_int64), ctypes.c_size_t]
    lib.axon_start_nrt_profile.restype = ctypes.c_int64
    lib.axon_stop_nrt_profile.argtypes = [ctypes.c_char_p]
    lib.axon_stop_nrt_profile.restype = ctypes.c_int64

    @contextlib.contextmanager
    def _hook(output_dir, device_ids):
        import jax
        jax.devices()
        if device_ids:
            ids = (ctypes.c_int64 * len(device_ids))(*device_ids)
            rc = lib.axon_start_nrt_profile(ids, len(device_ids))
        else:
            rc = lib.axon_start_nrt_profile(None, 0)
        if rc != 0:
            raise RuntimeError(f"axon_start_nrt_profile rc={rc}")
        try:
            yield
        finally:
            n = lib.axon_stop_nrt_profile(str(output_dir).encode())
            print(f"ntff: {n} profile file(s) in {output_dir}", file=sys.stderr)

    import antenv
    mod = types.ModuleType("antenv.axon_hooks")
    mod.get_axon_ntff_profile_hook = lambda: _hook
    mod.set_axon_ntff_profile_hook = lambda h: None
    sys.modules["antenv.axon_hooks"] = mod
    antenv.axon_hooks = mod


def kernel(x, mask, Wq, bq, Wk, bk, Wv, bv, Wo, bo, _trace=False,
           _n_dve_exp=0):
    global LAST_EXEC_NS
    x = np.ascontiguousarray(np.asarray(x, dtype=np.float32))
    mask = np.asarray(mask)
    Wq = np.asarray(Wq, dtype=np.float32)
    Wk = np.asarray(Wk, dtype=np.float32)
    Wv = np.asarray(Wv, dtype=np.float32)
    Wo = np.asarray(Wo, dtype=np.float32)
    bq = np.asarray(bq, dtype=np.float32)
    bk = np.asarray(bk, dtype=np.float32)
    bv = np.asarray(bv, dtype=np.float32)
    bo = np.asarray(bo, dtype=np.float32)

    scale = np.float32(1.0 / math.sqrt(DK))
    apply_mask = not bool((mask != 0).all())
    qkv_bias = bool(bq.any() or bk.any() or bv.any())

    nc = _build(apply_mask, qkv_bias, n_dve_exp=_n_dve_exp)

    if apply_mask:
        mbias = np.where(mask == 0, np.float32(-1e9), np.float32(0.0))
        # maskT[b][k, q] = mbias[b][q, k]
        maskT = np.ascontiguousarray(np.transpose(mbias, (0, 2, 1)))

    in_maps = []
    for b in range(B):
        xT_np = np.ascontiguousarray(x[b].T).astype(ml_dtypes.bfloat16)  # [D, S]
        for g in range(HG):
            rows = slice(g * HD, (g + 1) * HD)
            wqkv_np = np.ascontiguousarray(np.concatenate(
                [Wq[rows].T * scale, Wk[rows].T, Wv[rows].T],
                axis=1)).astype(ml_dtypes.bfloat16)
            wo_np = np.ascontiguousarray(
                Wo[:, rows].T).astype(ml_dtypes.bfloat16)
            m = {"xT": xT_np, "wqkv": wqkv_np, "wo": wo_np}
            if apply_mask:
                m["maskT"] = maskT[b]
            if qkv_bias:
                m["qkb"] = np.ascontiguousarray(
                    np.stack([bq[rows] * scale, bk[rows]]))
                m["vb"] = np.ascontiguousarray(bv[rows])
            in_maps.append(m)

    if _trace:
        _install_ntff_shim()
    r = run_bass_kernel_spmd(nc, in_maps, list(range(NCORES)), trace=_trace)
    LAST_EXEC_NS = r.exec_time_ns

    y = np.empty((B, S, D), dtype=np.float32)
    for b in range(B):
        yT = r.results[2 * b]["yT"] + r.results[2 * b + 1]["yT"]
        y[b] = yT.T + bo[None, :]
    return y
